# revision 37
# baseline (speedup 1.0000x reference)
"""Batched DWI 3D conv as implicit GEMM on 8 TRN2 NeuronCores.

Problem: x [8, 12, 12, 32, 32, 32] f32, W [32, 12, 12, 3, 3, 3] f32
         -> out [8, 32, 30, 30, 30] f32  (VALID 3D conv, c_in = 144)

Strategy (data-parallel over batch, one batch element per core):
  - x viewed as [144, 32, 32, 32] in SBUF; a kernel offset (dx, dy, dz)
    is a pure free-dim shift, so the conv is a chain of shifted matmuls
    accumulated in PSUM: out[co, n] += W_d^T @ x[:, n + shift(d)].
  - c_out = 32 fills only 1/4 of the PE array columns, so 4 col-tiled
    matmuls run concurrently (tile_position=(0, 32j)), each computing a
    different output chunk into its own 32-partition PSUM slice. Warm
    phase period measured 190 ns (= 450 cyc @ 2.4 GHz + NX issue); taps
    with dz=1 pay +12 ns (rhs base 2-byte misaligned; a fix needs a
    z-shifted second x copy = +8 MB/core DMA, net loss). NOTE: the chip
    sometimes sits in the P0 power state (PE ~2.0 GHz, phases 228 ns,
    total ~129 us) - run-to-run compares are invalid across states.
  - c_in = 144 = 128 + 16. Body: 27 ctile phases (K=128, AP shifts).
    Tail: the leftover 16 channels x 27 taps = 432 rows ride TWO
    host-built [128, 32^3] fp8 tiles of 8 shifted copies each (buffer A:
    dx=0 block minus the (2,2) corner; buffer B: dx=2 block plus both
    corners); 2 translates of each cover all 27 taps in 4 phases =
    31 phases/bank total, the exact floor ceil(144*27/128). fp8 e4m3
    tail adds ~1.2e-2 rel err - under the 2e-2 gate.
  - 31 phases/bank x 15 banks across groups sized [4,8,12,12,8,8,4,4]
    (banks [1,2,3,3,2,2,1,1]): ANY 3 consecutive groups fit in 8 PSUM
    banks, so a group's body never waits on the grandparent's cast
    (measured ~2.2 us of stalls with the old [1,2,1,4,4,2,1] layout).
    Deferred tails: group g's 4 tail phases execute after group g+1's
    full body; g's banks then cast to bf16 (DVE) and store.
  - Output stored as bf16 (halves store traffic; +1e-3 rel err).
  - Two HWDGE rings: input x slabs + tails on the sync ring in deadline
    order; WF's first slice/plane 1/all out stores on the scalar ring
    (parallel descriptor gen, ~0.6 us per dma_start each). Input is
    16.5 MB/core at ~350 GB/s effective - it saturates the queues until
    ~55 us; 4-plane DMA pieces interleaved by deadline keep the queue
    backlog balanced (8-plane pieces left single-queue backlogs whose
    records landed ~1.3 us past their deadline mid-stream).
  - 15 dummy matmuls warm the PE from engine-release (~7.3 us) to
    data-ready (~12 us): the HAM clock gate needs a FULL 3.4 us busy
    window to unthrottle 1.2->2.4 GHz, and any idle gap before the real
    stream restarts that clock, costing ~8 cold phases (+1.5 us). The
    warm chain self-limits (later warmups run warm at 190 ns).
Timeline (2.4 GHz state): ~7.3 us NEFF preamble (fixed) + ~4.8 us
warmup/data-wait + ~90.5 us stream + ~5.5 us final cast/store/teardown
= ~108.5 us (was 112.1 us at session start).
"""

import numpy as np
import ml_dtypes

import concourse.bass as bass
import concourse.bacc as bacc
import concourse.mybir as mybir
import concourse.tile as tile
from concourse.bass_utils import run_bass_kernel_spmd

BF16 = mybir.dt.bfloat16
FP8 = mybir.dt.float8e4
F32 = mybir.dt.float32

N_CORES = 8
CIN = 144
COUT = 32
DIM = 32
ODIM = 30
NCH = 450  # one chunk = 15 y-rows x 30 z
NFLAT = DIM * DIM * DIM
CHUNKS = [(ox, h) for ox in range(ODIM) for h in (0, 1)]  # 60 chunks
# chunks per group; ANY 3 consecutive groups' bank counts must sum to <= 8
# PSUM banks (g's body claims banks while g-1 is open and g-2 is casting)
GROUP_SIZES = [4, 8, 12, 12, 8, 8, 4, 4]
# x body loaded in plane slabs sized to stay ahead of PE consumption
XSLABS = [(0, 2), (2, 4), (4, 8), (8, 16), (16, 24), (24, 32)]

# tail cover: tap = block + translate, TWO baked buffers x 2 translates each
# = 4 phases (the single-buffer optimum is 5; 31 total phases/bank is the
# floor: ceil(144*27/128) = 31). Buffer A covers planes dx=0,1 minus the
# (2,2) corners; buffer B covers plane dx=2 plus both corners.
TAIL_A_BLOCKS = [(0, 0, 0), (0, 0, 1), (0, 0, 2), (0, 1, 0),
                 (0, 1, 1), (0, 1, 2), (0, 2, 0), (0, 2, 1)]
TAIL_B_BLOCKS = [(2, 0, 0), (2, 0, 1), (2, 0, 2), (2, 2, 0),
                 (2, 2, 1), (2, 2, 2), (0, 2, 2), (1, 2, 2)]
TAIL_BLOCKS = [TAIL_A_BLOCKS, TAIL_B_BLOCKS]
# (buffer index, free-dim translate (tx, ty, tz))
TAIL_PHASES = [(0, (0, 0, 0)), (0, (1, 0, 0)), (1, (0, 0, 0)), (1, (0, 1, 0))]
NTAIL = len(TAIL_PHASES)
N_CTILES = 27 + NTAIL
N_WARM = 13


def _tail_assign():
    """tap -> (buffer, block, phase), first-match; every tap covered once."""
    assign = {}
    for i, (buf, t) in enumerate(TAIL_PHASES):
        for b, s in enumerate(TAIL_BLOCKS[buf]):
            tap = (s[0] + t[0], s[1] + t[1], s[2] + t[2])
            if all(0 <= v <= 2 for v in tap) and tap not in assign:
                assign[tap] = (buf, b, i)
    assert len(assign) == 27, len(assign)
    return assign


_CACHE = {}


def build_nc():
    nc = bacc.Bacc(None, target_bir_lowering=False)
    xin = nc.dram_tensor("x", [128, DIM, DIM, DIM], BF16, kind="ExternalInput")
    xta_d = nc.dram_tensor("xta", [128, DIM, DIM, DIM], FP8,
                           kind="ExternalInput")
    xtb_d = nc.dram_tensor("xtb", [128, DIM, DIM, DIM], FP8,
                           kind="ExternalInput")
    wf_d = nc.dram_tensor("wf", [128, 27, COUT], BF16, kind="ExternalInput")
    wt_d = nc.dram_tensor("wt", [128, NTAIL, COUT], FP8, kind="ExternalInput")
    n_banks_total = sum(g // 4 for g in GROUP_SIZES)
    # [partition = 32*colgroup + co, bank_seq, 450] bf16
    out_d = nc.dram_tensor("out", [128, n_banks_total, NCH], BF16,
                           kind="ExternalOutput")

    with tile.TileContext(nc) as tc:
        with (
            tc.tile_pool(name="wpool", bufs=1) as wpool,
            tc.tile_pool(name="xpool", bufs=1) as xpool,
            tc.tile_pool(name="tpool", bufs=1) as tpool,
            tc.tile_pool(name="spool", bufs=3) as spool,
            tc.tile_pool(name="ppool", bufs=8, space="PSUM") as ppool,
        ):
            WF = wpool.tile([128, 27, COUT], BF16, tag="wf")
            WT = wpool.tile([128, NTAIL, COUT], FP8, tag="wt")
            XPG = [xpool.tile([128, p1 - p0, DIM, DIM], BF16, tag=f"xp{si}",
                              name=f"xp{si}")
                   for si, (p0, p1) in enumerate(XSLABS)]
            TA = tpool.tile([128, DIM, DIM, DIM], FP8, tag="tailA")
            TB = tpool.tile([128, DIM, DIM, DIM], FP8, tag="tailB")
            TBUF = [TA, TB]

            def load_slab(si, a=None, b=None):
                p0, p1 = XSLABS[si]
                if a is not None:
                    nc.sync.dma_start(XPG[si][:, a - p0: b - p0, :, :],
                                      xin[:, a:b, :, :])
                else:
                    nc.sync.dma_start(XPG[si][:], xin[:, p0:p1, :, :])

            def load_tail(a, b):
                nc.sync.dma_start(TA[:, a:b, :, :], xta_d[:, a:b, :, :])
                nc.sync.dma_start(TB[:, a:b, :, :], xtb_d[:, a:b, :, :])

            # issue order = completion order = deadline order (measured:
            # effective input rate ~290GB/s; group g's tail runs after group
            # g+1's body; first tail phases land ~29us in)
            # first phases need only the dx=0 weight slice and planes 0-1;
            # spread those first loads across BOTH HWDGE rings (sync+scalar)
            # so their descriptor gens run in parallel -> earlier data-ready.
            # Only small volumes ride the scalar ring (no queue flooding).
            # biggest first-phase dependency (plane 1) leads the scalar ring
            # so its descriptor gen doesn't queue behind the small WF slice
            nc.sync.dma_start(XPG[0][:, 0, :, :], xin[:, 0, :, :])
            nc.scalar.dma_start(XPG[0][:, 1, :, :], xin[:, 1, :, :])
            nc.scalar.dma_start(WF[:, 0:9, :], wf_d[:, 0:9, :])
            nc.sync.dma_start(WF[:, 9:27, :], wf_d[:, 9:27, :])
            nc.sync.dma_start(WT[:], wt_d[:])
            # 4-plane pieces, interleaved by deadline: finer records spread
            # the queue backlog evenly so the drain is less imbalanced (the
            # old 8-plane slabs left single-queue backlogs that landed
            # ~1.3us late at 53-57us)
            load_slab(1)          # planes 2-3    (~18us)
            load_slab(2)          # planes 4-7    (~18us)
            load_tail(0, 4)       # tail 0-3   (g0 tail, ~29us)
            load_slab(3, 8, 12)   # planes 8-11   (~29us)
            load_slab(3, 12, 16)  # planes 12-15  (~31us)
            load_tail(4, 8)       # tail 4-7   (g1 tail, ~44us)
            load_slab(4, 16, 20)  # planes 16-19  (~51us)
            load_slab(4, 20, 24)  # planes 20-23  (~54us)
            load_tail(8, 12)      # tail 8-11  (g2 tail, ~62us)
            load_tail(12, 16)     # tail 12-15 (g2 tail, ~62us)
            load_slab(5, 24, 28)  # planes 24-27  (~76us)
            load_slab(5, 28, 32)  # planes 28-31  (~88us)
            load_tail(16, 20)     # tail 16-19 (g3 tail, ~74us)
            load_tail(20, 24)     # tail 20-23 (g4 tail, ~87us)
            load_tail(24, 28)     # tail 24-27 (g5 tail, ~93us)
            load_tail(28, 32)     # tail 28-31 (g6 tail, ~100us)

            # warm up the PE (HAM clock gate) during the initial load window;
            # memset on GpSimd: it is the first engine the runtime releases
            # (~5.8us), so the warm chain starts ~1us before any other engine
            warm = wpool.tile([128, 482], BF16, tag="warm")
            nc.gpsimd.memset(warm[:], 0.0)
            # warm chain: coarse N=450 pieces, then a fine N=180 tail so the
            # in-order PE queue's last warmup overshoots data-ready by <=75ns
            # on average (a 450-wide warmup can overshoot by up to 375ns)
            pwarm = ppool.tile([128, NCH], F32, tag="ps", name="ps_warm")
            warm_ns = [450] * (N_WARM - 4) + [180] * 4
            for wi, nw in enumerate(warm_ns):
                nc.tensor.matmul(pwarm[0:32, 0:nw], warm[:, 0:32],
                                 warm[:, 32:32 + nw],
                                 start=(wi == 0), stop=(wi == len(warm_ns) - 1),
                                 tile_position=(0, 0))

            def xplane(p):
                for si, (p0, p1) in enumerate(XSLABS):
                    if p < p1:
                        return XPG[si], p - p0
                raise AssertionError

            def emit_body(ptiles, gch, trange):
                for t in trange:
                    dx, dy, dz = t // 9, (t // 3) % 3, t % 3
                    lhsT = WF[:, t, :]
                    for bi in range(len(ptiles)):
                        P = ptiles[bi]
                        for j in range(4):
                            ox, h = gch[bi * 4 + j]
                            y0 = 15 * h
                            xt_, lp = xplane(ox + dx)
                            rhs = xt_[:, lp, y0 + dy: y0 + dy + 15, dz: dz + 30]
                            nc.tensor.matmul(
                                P[32 * j: 32 * (j + 1), :], lhsT, rhs,
                                start=(t == 0), stop=False,
                                tile_position=(0, 32 * j))

            def emit_tail_and_store(ptiles, gch, nb0, last=False):
                for i in range(NTAIL):
                    buf, (tx, ty, tz) = TAIL_PHASES[i]
                    T = TBUF[buf]
                    lhsT = WT[:, i, :]
                    for bi in range(len(ptiles)):
                        P = ptiles[bi]
                        for j in range(4):
                            ox, h = gch[bi * 4 + j]
                            y0 = 15 * h
                            rhs = T[:, ox + tx, y0 + ty: y0 + ty + 15,
                                    tz: tz + 30]
                            nc.tensor.matmul(
                                P[32 * j: 32 * (j + 1), :], lhsT, rhs,
                                start=False, stop=(i == NTAIL - 1),
                                tile_position=(0, 32 * j))
                nbank = len(ptiles)
                st = spool.tile([128, nbank * NCH], BF16, tag="st",
                                padded_shape=[128, 4 * NCH], name=f"st_{nb0}")
                if last:
                    # final store is on the critical path: cast+store in two
                    # pieces on different HWDGE rings so the descriptor gens
                    # run in parallel and DMA p1 overlaps the cast of p2.
                    # Unbalanced split: the bigger piece rides the earlier
                    # cast, balancing the two chains' finish times
                    h = 275
                    nc.vector.tensor_copy(st[:, 0:h], ptiles[0][:, 0:h])
                    nc.scalar.dma_start(out_d[:, nb0, 0:h], st[:, 0:h])
                    nc.vector.tensor_copy(st[:, h:NCH], ptiles[0][:, h:NCH])
                    nc.sync.dma_start(out_d[:, nb0, h:NCH], st[:, h:NCH])
                    return
                for bi in range(nbank):
                    nc.vector.tensor_copy(st[:, bi * NCH: (bi + 1) * NCH],
                                          ptiles[bi][:])
                # stores ride the scalar ring: keeps the sync ring free for
                # input descriptor generation mid-stream
                nc.scalar.dma_start(out_d[:, nb0: nb0 + nbank, :], st[:])

            g0 = 0
            nb0 = 0
            pending = None  # (ptiles, gch, nb0) awaiting tail+store
            for gi, gsz in enumerate(GROUP_SIZES):
                gch = CHUNKS[g0: g0 + gsz]
                nbank = gsz // 4
                ptiles = [ppool.tile([128, NCH], F32, tag="ps",
                                     name=f"ps_{gi}_{bi}")
                          for bi in range(nbank)]
                emit_body(ptiles, gch, range(0, 27))
                if pending is not None:
                    emit_tail_and_store(*pending)
                pending = (ptiles, gch, nb0)
                g0 += gsz
                nb0 += nbank
            emit_tail_and_store(*pending, last=True)

    nc.compile()
    return nc


def _get_nc():
    if "nc" not in _CACHE:
        _CACHE["nc"] = build_nc()
    return _CACHE["nc"]


def _prep_inputs(x, W):
    bf16 = ml_dtypes.bfloat16
    fp8 = ml_dtypes.float8_e4m3
    xr = np.asarray(x).reshape(8, CIN, DIM, DIM, DIM)
    Wr = np.asarray(W).reshape(COUT, CIN, 3, 3, 3).astype(np.float32)

    xb = np.ascontiguousarray(xr[:, :128]).astype(bf16)

    # host-built shifted fp8 tails: in each buffer, block b holds the 16
    # tail channels shifted left by flat(block[b]) in 32^3 space, zero-filled
    tails = np.ascontiguousarray(xr[:, 128:144]).reshape(8, 16, NFLAT)
    xts = []
    for blocks in TAIL_BLOCKS:
        xt = np.zeros((8, 128, NFLAT), fp8)
        for b, (sx, sy, sz) in enumerate(blocks):
            s = sx * DIM * DIM + sy * DIM + sz
            r = b * 16
            xt[:, r: r + 16, 0: NFLAT - s] = tails[:, :, s:].astype(fp8)
        xts.append(xt.reshape(8, 128, DIM, DIM, DIM))
    xta, xtb = xts

    wf = np.ascontiguousarray(
        Wr[:, :128].reshape(COUT, 128, 27).transpose(1, 2, 0)
    ).astype(bf16)

    # tail weights: row b*16+c, phase i gets W[co, 128+c, block+translate]
    # if that tap is assigned to (buf, b, i), else 0 (each tap covered once)
    assign = _tail_assign()
    wt = np.zeros((128, NTAIL, COUT), np.float32)
    tailW = Wr[:, 128:144]  # [co, c, dx, dy, dz]
    for tap, (buf, b, i) in assign.items():
        dx, dy, dz = tap
        r = b * 16
        wt[r: r + 16, i, :] = tailW[:, :, dx, dy, dz].T
    wt = wt.astype(fp8)

    return [{"x": xb[b], "xta": xta[b], "xtb": xtb[b], "wf": wf, "wt": wt}
            for b in range(N_CORES)]


def kernel(x, W, _trace=False):
    nc = _get_nc()
    in_maps = _prep_inputs(np.asarray(x), np.asarray(W))
    res = None
    for attempt in range(3):
        try:
            res = run_bass_kernel_spmd(nc, in_maps, list(range(N_CORES)),
                                       trace=_trace)
            break
        except Exception:
            # rare transient NRT_EXEC_UNIT_UNRECOVERABLE flakes; retry
            if attempt == 2:
                raise
            import time as _time
            _time.sleep(2.0)
    full = np.empty((N_CORES, COUT, ODIM, ODIM, ODIM), np.float32)
    for b in range(N_CORES):
        o = np.asarray(res.results[b]["out"]).astype(np.float32)
        nb = 0
        g0 = 0
        for gsz in GROUP_SIZES:
            for bi in range(gsz // 4):
                for j in range(4):
                    ox, h = CHUNKS[g0 + 4 * bi + j]
                    full[b, :, ox, 15 * h: 15 * h + 15, :] = (
                        o[32 * j: 32 * j + 32, nb].reshape(COUT, 15, 30))
                nb += 1
            g0 += gsz
    if _trace:
        return full, res
    return full

